# revision 9
# baseline (speedup 1.0000x reference)
"""Bass/Trainium2 kernel for DropConnect (training path, Wstd != 0).

Z[b,o] = sum_i X[b,i] * W[i,o] * Werr[loc_id[b],i,o] + bias[o] * Berr[loc_id[b],o]

Strategy (8 NeuronCores, data-parallel over batch):
  - each core handles 16 samples; W and the Werr pool are replicated.
    bias*Berr[loc] is precomputed on the host (loc_id is host-visible) and
    shipped as a flat [1, 16*512] f32 row, so no Berr gather and no bias
    matmul happen on device.
  - per sample, the 1MB Werr[loc] slab is gathered on-device with one indirect
    DMA that CASTS f32 -> bf16 in flight (software-DGE feature): Werr viewed
    as [128000, 2048] macro-rows, dest partition p pulls the contiguous 8KB
    macro-row loc*128+p (input rows i=4p..4p+3). All 16 gathers are issued
    back-to-back upfront on gpsimd so the 16 DMA engines never starve; the
    first four go ahead of the small cast-loads to start the pipeline early.
  - VectorE computes the bf16 W*Werr product per sample ([128,2048] 16-bit
    tensor_tensor at 2x DVE rate), and also evicts each sample's PSUM row
    with a fused tensor_add against the membias row (bias add + eviction in
    one [1,512] op), emitted with a one-sample lag so the adds don't
    serialize the TT pipeline against TensorE.
  - TensorE contracts with X: 4 matmuls of [128,1]x[128,512] bf16 into a
    [1,512] PSUM tile per sample. Output ships in two halves.
"""

import sys

sys.path.insert(0, "/opt/trn_rl_repo")

import numpy as np

B, IN, OUT, POOL, NCORES = 128, 512, 512, 1000, 8
BL = B // NCORES  # samples per core
WT_COLS = 4 * OUT  # 2048: one macro-row = 4 input rows of W/Werr

_CACHE = {}


def _build(pool_entries=POOL):
    import concourse.bass as bass
    import concourse.mybir as mybir
    import concourse.tile as tile
    from concourse import bacc

    f32, i32, bf16 = mybir.dt.float32, mybir.dt.int32, mybir.dt.bfloat16

    nc = bacc.Bacc("TRN2", debug=False)
    werr = nc.dram_tensor(
        "Werr", [pool_entries * 128, WT_COLS], f32, kind="ExternalInput"
    )
    wr = nc.dram_tensor("Wr", [128, WT_COLS], f32, kind="ExternalInput")
    xt = nc.dram_tensor("Xt", [128, BL * 4], f32, kind="ExternalInput")
    idx = nc.dram_tensor("idx", [128, BL], i32, kind="ExternalInput")
    memb = nc.dram_tensor("memb", [1, BL * OUT], f32, kind="ExternalInput")
    z = nc.dram_tensor("Z", [1, BL * OUT], f32, kind="ExternalOutput")

    with tile.TileContext(nc) as tc:
        with (
            tc.tile_pool(name="const", bufs=1) as cpool,
            tc.tile_pool(name="wts", bufs=BL) as wpool,
            tc.tile_pool(name="prod", bufs=4) as ptpool,
            tc.tile_pool(name="ps", bufs=8, space="PSUM") as ppool,
        ):
            # idx first: the Werr gathers are gated only on this tiny load
            idx_sb = cpool.tile([128, BL], i32)
            nc.sync.dma_start(idx_sb[:], idx.ap())
            memb_sb = cpool.tile([1, BL * OUT], f32)
            nc.sync.dma_start(memb_sb[:], memb.ap())

            def gather(b):
                wt = wpool.tile([128, WT_COLS], bf16, tag="wt")
                nc.gpsimd.indirect_dma_start(
                    out=wt[:],
                    out_offset=None,
                    in_=werr.ap(),
                    in_offset=bass.IndirectOffsetOnAxis(
                        ap=idx_sb[:, b : b + 1], axis=0
                    ),
                )
                return wt

            # all 16 gather issues go first on gpsimd so the DMA engines
            # are fed as early as possible
            wts = [gather(b) for b in range(BL)]

            # W / Xt load as f32 on the Sync HWDGE queue and are cast to
            # bf16 by VectorE (idle until the first product anyway)
            wrf_sb = cpool.tile([128, WT_COLS], f32)
            nc.sync.dma_start(wrf_sb[:], wr.ap())
            xtf_sb = cpool.tile([128, BL * 4], f32)
            nc.sync.dma_start(xtf_sb[:], xt.ap())
            wr_sb = cpool.tile([128, WT_COLS], bf16)
            nc.vector.tensor_copy(wr_sb[:], wrf_sb[:])
            xt_sb = cpool.tile([128, BL * 4], bf16)
            nc.vector.tensor_copy(xt_sb[:], xtf_sb[:])
            zstage = cpool.tile([1, BL * OUT], f32)

            prev = None  # (ps tile, sample index) awaiting eviction
            for b in range(BL):
                wt = wts[b]
                pt = ptpool.tile([128, WT_COLS], bf16, tag="pt")
                nc.vector.tensor_mul(pt[:], wt[:], wr_sb[:])
                ps = ppool.tile([1, OUT], f32, tag="ps")
                for j in range(4):
                    nc.tensor.matmul(
                        out=ps[:],
                        lhsT=xt_sb[:, 4 * b + j : 4 * b + j + 1],
                        rhs=pt[:, j * OUT : (j + 1) * OUT],
                        start=(j == 0),
                        stop=(j == 3),
                    )
                if prev is not None:
                    pb = prev[1]
                    nc.vector.tensor_add(
                        zstage[0:1, pb * OUT : (pb + 1) * OUT],
                        prev[0][:],
                        memb_sb[0:1, pb * OUT : (pb + 1) * OUT],
                    )
                    if pb == BL // 2 - 1:
                        # first half of the output ships while the second
                        # half is still being computed
                        nc.sync.dma_start(
                            z.ap()[:, : (BL // 2) * OUT],
                            zstage[0:1, : (BL // 2) * OUT],
                        )
                prev = (ps, b)

            pb = prev[1]
            nc.vector.tensor_add(
                zstage[0:1, pb * OUT : (pb + 1) * OUT],
                prev[0][:],
                memb_sb[0:1, pb * OUT : (pb + 1) * OUT],
            )
            nc.sync.dma_start(
                z.ap()[:, (BL // 2) * OUT :], zstage[0:1, (BL // 2) * OUT :]
            )

    nc.compile()
    return nc


def get_nc(pool_entries=POOL):
    key = ("nc", pool_entries)
    if key not in _CACHE:
        _CACHE[key] = _build(pool_entries)
    return _CACHE[key]


def make_in_maps(X, W, bias, Werr, Berr, loc_id):
    X = np.ascontiguousarray(np.asarray(X, dtype=np.float32))
    W = np.ascontiguousarray(np.asarray(W, dtype=np.float32))
    bias = np.ascontiguousarray(np.asarray(bias, dtype=np.float32))
    Werr = np.ascontiguousarray(np.asarray(Werr, dtype=np.float32))
    Berr = np.ascontiguousarray(np.asarray(Berr, dtype=np.float32))
    loc_id = np.ascontiguousarray(np.asarray(loc_id, dtype=np.int32))

    pool_entries = Werr.shape[0]
    werr2d = Werr.reshape(pool_entries * 128, WT_COLS)
    wr = W.reshape(128, WT_COLS)
    p_iota = np.arange(128, dtype=np.int32)[:, None]

    in_maps = []
    for c in range(NCORES):
        xc = X[c * BL : (c + 1) * BL]  # [BL, IN]
        locc = loc_id[c * BL : (c + 1) * BL]  # [BL]
        xt = np.ascontiguousarray(
            xc.reshape(BL, 128, 4).transpose(1, 0, 2).reshape(128, BL * 4)
        )
        idxc = np.ascontiguousarray(locc[None, :] * 128 + p_iota).astype(np.int32)
        membc = np.ascontiguousarray(
            (bias[None, :] * Berr[locc]).reshape(1, BL * OUT)
        )
        in_maps.append(
            {
                "Werr": werr2d,
                "Wr": wr,
                "Xt": xt,
                "idx": idxc,
                "memb": membc,
            }
        )
    return in_maps


def _reset_accelerator():
    import ctypes

    try:
        lib = ctypes.CDLL("/opt/axon/libaxon_pjrt.so")
        lib.axon_reset.restype = ctypes.c_int64
        lib.axon_reset()
    except Exception:
        pass


def kernel(X, W, bias, Werr, Berr, loc_id):
    from concourse.bass_utils import run_bass_kernel_spmd

    nc = get_nc()
    in_maps = make_in_maps(X, W, bias, Werr, Berr, loc_id)
    try:
        res = run_bass_kernel_spmd(nc, in_maps, core_ids=list(range(NCORES)))
    except Exception:
        # a wedged NeuronCore surfaces as an unrecoverable-device error;
        # reset the accelerator once and retry
        _reset_accelerator()
        res = run_bass_kernel_spmd(nc, in_maps, core_ids=list(range(NCORES)))
    out = np.concatenate(
        [res.results[c]["Z"].reshape(BL, OUT) for c in range(NCORES)], axis=0
    )
    return out


# revision 10
# speedup vs baseline: 1.1355x; 1.1355x over previous
"""Bass/Trainium2 kernel for DropConnect (training path, Wstd != 0).

Z[b,o] = sum_i X[b,i] * W[i,o] * Werr[loc_id[b],i,o] + bias[o] * Berr[loc_id[b],o]

Strategy (8 NeuronCores, data-parallel over batch):
  - each core handles 16 samples; W and the Werr pool are replicated.
    bias*Berr[loc] is precomputed on the host (loc_id is host-visible) and
    shipped as a flat [1, 16*512] f32 row, so no Berr gather and no bias
    matmul happen on device.
  - per sample, the 1MB Werr[loc] slab is gathered on-device with one indirect
    DMA that CASTS f32 -> bf16 in flight (software-DGE feature): Werr viewed
    as [128000, 2048] macro-rows, dest partition p pulls the contiguous 8KB
    macro-row loc*128+p (input rows i=4p..4p+3). All 16 gathers are issued
    back-to-back upfront on gpsimd so the 16 DMA engines never starve; the
    first four go ahead of the small cast-loads to start the pipeline early.
  - VectorE computes the bf16 W*Werr product per sample ([128,2048] 16-bit
    tensor_tensor at 2x DVE rate), and also evicts each sample's PSUM row
    with a fused tensor_add against the membias row (bias add + eviction in
    one [1,512] op), emitted with a one-sample lag so the adds don't
    serialize the TT pipeline against TensorE.
  - TensorE contracts with X: 4 matmuls of [128,1]x[128,512] bf16 into a
    [1,512] PSUM tile per sample. Output ships in two halves.
"""

import sys

sys.path.insert(0, "/opt/trn_rl_repo")

import numpy as np

B, IN, OUT, POOL, NCORES = 128, 512, 512, 1000, 8
BL = B // NCORES  # samples per core
WT_COLS = 4 * OUT  # 2048: one macro-row = 4 input rows of W/Werr

_CACHE = {}


def _build(pool_entries=POOL):
    import concourse.bass as bass
    import concourse.mybir as mybir
    import concourse.tile as tile
    from concourse import bacc

    f32, i32, bf16 = mybir.dt.float32, mybir.dt.int32, mybir.dt.bfloat16

    nc = bacc.Bacc("TRN2", debug=False)
    werr = nc.dram_tensor(
        "Werr", [pool_entries * 128, WT_COLS], f32, kind="ExternalInput"
    )
    wr = nc.dram_tensor("Wr", [128, WT_COLS], f32, kind="ExternalInput")
    xt = nc.dram_tensor("Xt", [128, BL * 4], f32, kind="ExternalInput")
    idx = nc.dram_tensor("idx", [128, BL], i32, kind="ExternalInput")
    memb = nc.dram_tensor("memb", [1, BL * OUT], f32, kind="ExternalInput")
    z = nc.dram_tensor("Z", [1, BL * OUT], f32, kind="ExternalOutput")

    EARLY = 4  # gathers issued before the small cast-loads

    with tile.TileContext(nc) as tc:
        with (
            tc.tile_pool(name="const", bufs=1) as cpool,
            tc.tile_pool(name="wts", bufs=BL) as wpool,
            tc.tile_pool(name="prod", bufs=4) as ptpool,
            tc.tile_pool(name="ps", bufs=8, space="PSUM") as ppool,
        ):
            # idx first: the Werr gathers are gated only on this tiny load
            idx_sb = cpool.tile([128, BL], i32)
            nc.sync.dma_start(idx_sb[:], idx.ap())
            memb_sb = cpool.tile([1, BL * OUT], f32)
            nc.sync.dma_start(memb_sb[:], memb.ap())

            def gather(b):
                wt = wpool.tile([128, WT_COLS], bf16, tag="wt")
                nc.gpsimd.indirect_dma_start(
                    out=wt[:],
                    out_offset=None,
                    in_=werr.ap(),
                    in_offset=bass.IndirectOffsetOnAxis(
                        ap=idx_sb[:, b : b + 1], axis=0
                    ),
                )
                return wt

            wts = [gather(b) for b in range(EARLY)]

            # small bf16 cast-loads on the gpsimd software DGE
            wr_sb = cpool.tile([128, WT_COLS], bf16)
            nc.gpsimd.dma_start(out=wr_sb[:], in_=wr.ap())
            xt_sb = cpool.tile([128, BL * 4], bf16)
            nc.gpsimd.dma_start(out=xt_sb[:], in_=xt.ap())
            zstage = cpool.tile([1, BL * OUT], f32)

            wts += [gather(b) for b in range(EARLY, BL)]

            prev = None  # (ps tile, sample index) awaiting eviction
            for b in range(BL):
                wt = wts[b]
                pt = ptpool.tile([128, WT_COLS], bf16, tag="pt")
                nc.vector.tensor_mul(pt[:], wt[:], wr_sb[:])
                ps = ppool.tile([1, OUT], f32, tag="ps")
                for j in range(4):
                    nc.tensor.matmul(
                        out=ps[:],
                        lhsT=xt_sb[:, 4 * b + j : 4 * b + j + 1],
                        rhs=pt[:, j * OUT : (j + 1) * OUT],
                        start=(j == 0),
                        stop=(j == 3),
                    )
                if prev is not None:
                    pb = prev[1]
                    nc.vector.tensor_add(
                        zstage[0:1, pb * OUT : (pb + 1) * OUT],
                        prev[0][:],
                        memb_sb[0:1, pb * OUT : (pb + 1) * OUT],
                    )
                    if pb == BL // 2 - 1:
                        # first half of the output ships while the second
                        # half is still being computed
                        nc.sync.dma_start(
                            z.ap()[:, : (BL // 2) * OUT],
                            zstage[0:1, : (BL // 2) * OUT],
                        )
                prev = (ps, b)

            pb = prev[1]
            nc.vector.tensor_add(
                zstage[0:1, pb * OUT : (pb + 1) * OUT],
                prev[0][:],
                memb_sb[0:1, pb * OUT : (pb + 1) * OUT],
            )
            nc.sync.dma_start(
                z.ap()[:, (BL // 2) * OUT :], zstage[0:1, (BL // 2) * OUT :]
            )

    nc.compile()
    return nc


def get_nc(pool_entries=POOL):
    key = ("nc", pool_entries)
    if key not in _CACHE:
        _CACHE[key] = _build(pool_entries)
    return _CACHE[key]


def make_in_maps(X, W, bias, Werr, Berr, loc_id):
    X = np.ascontiguousarray(np.asarray(X, dtype=np.float32))
    W = np.ascontiguousarray(np.asarray(W, dtype=np.float32))
    bias = np.ascontiguousarray(np.asarray(bias, dtype=np.float32))
    Werr = np.ascontiguousarray(np.asarray(Werr, dtype=np.float32))
    Berr = np.ascontiguousarray(np.asarray(Berr, dtype=np.float32))
    loc_id = np.ascontiguousarray(np.asarray(loc_id, dtype=np.int32))

    pool_entries = Werr.shape[0]
    werr2d = Werr.reshape(pool_entries * 128, WT_COLS)
    wr = W.reshape(128, WT_COLS)
    p_iota = np.arange(128, dtype=np.int32)[:, None]

    in_maps = []
    for c in range(NCORES):
        xc = X[c * BL : (c + 1) * BL]  # [BL, IN]
        locc = loc_id[c * BL : (c + 1) * BL]  # [BL]
        xt = np.ascontiguousarray(
            xc.reshape(BL, 128, 4).transpose(1, 0, 2).reshape(128, BL * 4)
        )
        idxc = np.ascontiguousarray(locc[None, :] * 128 + p_iota).astype(np.int32)
        membc = np.ascontiguousarray(
            (bias[None, :] * Berr[locc]).reshape(1, BL * OUT)
        )
        in_maps.append(
            {
                "Werr": werr2d,
                "Wr": wr,
                "Xt": xt,
                "idx": idxc,
                "memb": membc,
            }
        )
    return in_maps


def _reset_accelerator():
    import ctypes

    try:
        lib = ctypes.CDLL("/opt/axon/libaxon_pjrt.so")
        lib.axon_reset.restype = ctypes.c_int64
        lib.axon_reset()
    except Exception:
        pass


def kernel(X, W, bias, Werr, Berr, loc_id):
    from concourse.bass_utils import run_bass_kernel_spmd

    nc = get_nc()
    in_maps = make_in_maps(X, W, bias, Werr, Berr, loc_id)
    try:
        res = run_bass_kernel_spmd(nc, in_maps, core_ids=list(range(NCORES)))
    except Exception:
        # a wedged NeuronCore surfaces as an unrecoverable-device error;
        # reset the accelerator once and retry
        _reset_accelerator()
        res = run_bass_kernel_spmd(nc, in_maps, core_ids=list(range(NCORES)))
    out = np.concatenate(
        [res.results[c]["Z"].reshape(BL, OUT) for c in range(NCORES)], axis=0
    )
    return out
